# revision 1
# baseline (speedup 1.0000x reference)
"""Trainium2 Bass kernel for nn_Embedding2Score (segment_reduce), fp16 rev.

Strategy (data-parallel over sessions, per sharding hint):
  - 4096 graphs -> 8 cores x 512 graphs (4 blocks of 128 graphs each).
    Each core owns whole contiguous segments (batch is sorted by graph).
  - Everything is fp16 on the wire (inputs, weights, one-hots, scores);
    accumulation stays fp32 in PSUM. Halves HBM traffic vs fp32; sim'd
    end-to-end rel err ~8e-4 against the fp32 reference.
  - Segment broadcast (v_n -> nodes) and segment sum (alpha*x -> s_g) are
    one-hot matmuls on PE; one-hot blocks built with DVE is_equal against
    iota constants. alpha is folded into the one-hot (aS = alpha * S), so
    the segment-sum matmul consumes raw x directly.
  - Phase-2 scoring: s_h^T per 128-graph block vs item_weight^T in vocab
    groups of 10x500 cols (1.25 MB fp16 per DMA), PSUM->SBUF copies
    alternate DVE/ACT, stores are 1.25 MB each.
  - All loads ride the gpsimd SWDGE queue; y stores ride the SP HWDGE
    queue -> no head-of-line blocking between loads and stores.
  - Phase-1 and phase-2 PSUM pools coexist (8 banks total) so the Tile
    scheduler can overlap scoring of block b with phase 1 of block b+1.
"""

import sys

if "/opt/trn_rl_repo" not in sys.path:
    sys.path.insert(0, "/opt/trn_rl_repo")

import numpy as np

P = 128          # partitions / tile edge
D = 128          # hidden size
NCORES = 8
NBLK = 4         # graph blocks per core, 128 graphs each
BC = NBLK * P    # graphs per core = 512
VT = 500         # vocab tile (one PSUM bank)
VG = 10          # vocab tiles per group/DMA (5000 cols = 1.25 MB fp16)
ST = 4           # 128-node subtiles per supertile


def build_nc(ntpb, vpad, repeat=1, phase="both"):
    """Build the per-core Bass program. ntpb = node tiles per graph-block,
    vpad = padded vocab size (multiple of VT*VG). repeat>1 wraps the body
    in a hardware loop (timing probes). phase: 'both' | 'p1' | 'p2'."""
    import contextlib
    import concourse.bacc as bacc
    import concourse.mybir as mybir
    from concourse.tile import TileContext

    f16 = mybir.dt.float16
    f32 = mybir.dt.float32
    npb = ntpb * P
    nc = bacc.Bacc()

    xpk_ext = nc.declare_dram_parameter("xpk", [P, NBLK * npb], f16, isOutput=False)
    xtp_ext = nc.declare_dram_parameter("xtp", [P, NBLK * npb], f16, isOutput=False)
    blc_ext = nc.declare_dram_parameter("blc", [NBLK, P, ntpb], f32, isOutput=False)
    blr_ext = nc.declare_dram_parameter("blr", [NBLK, npb], f16, isOutput=False)
    vnt_ext = nc.declare_dram_parameter("vnt", [D, BC], f16, isOutput=False)
    w1t_ext = nc.declare_dram_parameter("w1t", [D, D], f16, isOutput=False)
    w2t_ext = nc.declare_dram_parameter("w2t", [D, D], f16, isOutput=False)
    w3at_ext = nc.declare_dram_parameter("w3at", [D, D], f16, isOutput=False)
    w3bt_ext = nc.declare_dram_parameter("w3bt", [D, D], f16, isOutput=False)
    b12c_ext = nc.declare_dram_parameter("b12c", [P, 1], f32, isOutput=False)
    w3bc_ext = nc.declare_dram_parameter("w3bc", [P, 1], f32, isOutput=False)
    qwt_ext = nc.declare_dram_parameter("qwt", [D, 1], f16, isOutput=False)
    qbc_ext = nc.declare_dram_parameter("qbc", [P, 1], f32, isOutput=False)
    itwt_ext = nc.declare_dram_parameter("itwt", [D, vpad], f16, isOutput=False)
    y_ext = nc.declare_dram_parameter("y", [BC, vpad], f16, isOutput=True)

    with TileContext(nc) as tc:
        with tc.tile_pool(name="const", bufs=1) as cp:
            iota_i = cp.tile([P, P], mybir.dt.int32, tag="iotai")
            nc.gpsimd.iota(iota_i[:], pattern=[[1, P]], base=0, channel_multiplier=0)
            iota_row = cp.tile([P, P], f16, tag="iotarow")
            nc.vector.tensor_copy(out=iota_row[:], in_=iota_i[:])
            iota_ci = cp.tile([P, 1], mybir.dt.int32, tag="iotaci")
            nc.gpsimd.iota(iota_ci[:], pattern=[[0, 1]], base=0, channel_multiplier=1)
            iota_col = cp.tile([P, 1], f32, tag="iotacol")
            nc.vector.tensor_copy(out=iota_col[:], in_=iota_ci[:])

            def load(name, ext, shape, dt=f16):
                t = cp.tile(shape, dt, tag=name)
                nc.gpsimd.dma_start(out=t[:], in_=ext[:])
                return t

            w1t = load("w1t", w1t_ext, [D, D])
            w2t = load("w2t", w2t_ext, [D, D])
            w3at = load("w3at", w3at_ext, [D, D])
            w3bt = load("w3bt", w3bt_ext, [D, D])
            b12c = load("b12c", b12c_ext, [P, 1], f32)
            w3bc = load("w3bc", w3bc_ext, [P, 1], f32)
            qwt = load("qwt", qwt_ext, [D, 1])
            qbc = load("qbc", qbc_ext, [P, 1], f32)
            vnt = load("vnt", vnt_ext, [D, BC])

            shT = cp.tile([D, BC], f16, tag="shT")  # s_h^T, filled per block
            if phase.startswith("p2"):
                nc.vector.memset(shT[:], 0.01)

            rep_ctx = tc.For_i(0, repeat, 1) if repeat > 1 else contextlib.nullcontext()
            with rep_ctx:
                _build_body(nc, tc, mybir, ntpb, vpad,
                            xpk_ext, xtp_ext, blc_ext, blr_ext, itwt_ext, y_ext,
                            iota_row, iota_col,
                            w1t, w2t, w3at, w3bt, b12c, w3bc, qwt, qbc, vnt, shT,
                            phase)

    nc.compile()
    return nc


def _build_body(nc, tc, mybir, ntpb, vpad,
                xpk_ext, xtp_ext, blc_ext, blr_ext, itwt_ext, y_ext,
                iota_row, iota_col,
                w1t, w2t, w3at, w3bt, b12c, w3bc, qwt, qbc, vnt, shT,
                phase="both"):
    f16 = mybir.dt.float16
    f32 = mybir.dt.float32
    npb = ntpb * P
    nst = -(-ntpb // ST)          # supertiles per block
    Sig = mybir.ActivationFunctionType.Sigmoid
    EQ = mybir.AluOpType.is_equal
    W = VG * VT
    ngrp = vpad // W
    p1 = phase in ("both", "p1")
    p2 = phase in ("both", "p2", "p2mm", "p2cp", "p2st")
    do_cp = phase not in ("p2mm", "p2st")   # PSUM->SBUF copies
    do_st = phase not in ("p2mm", "p2cp")   # y stores

    with tc.tile_pool(name="p1big", bufs=2) as pb, \
         tc.tile_pool(name="p1", bufs=4) as pool, \
         tc.tile_pool(name="blkp", bufs=2) as blkp, \
         tc.tile_pool(name="p2i", bufs=3) as p2i, \
         tc.tile_pool(name="p2o", bufs=4) as p2o, \
         tc.tile_pool(name="psPre", bufs=2, space="PSUM") as psPre, \
         tc.tile_pool(name="psBlk", bufs=1, space="PSUM") as psBlk, \
         tc.tile_pool(name="psAl", bufs=1, space="PSUM") as psAl, \
         tc.tile_pool(name="psSg", bufs=1, space="PSUM") as psSg, \
         tc.tile_pool(name="ps2", bufs=3, space="PSUM") as ps2:
        if p1:
            for blk in range(NBLK):
                gsl = slice(blk * P, (blk + 1) * P)
                nsl = slice(blk * npb, (blk + 1) * npb)
                xpk = pb.tile([P, npb], f16, tag="xpk")
                nc.gpsimd.dma_start(out=xpk[:], in_=xpk_ext[:, nsl])
                xtp = pb.tile([P, npb], f16, tag="xtp")
                nc.gpsimd.dma_start(out=xtp[:], in_=xtp_ext[:, nsl])
                blc = blkp.tile([P, ntpb], f32, tag="blc")
                nc.gpsimd.dma_start(out=blc[:], in_=blc_ext[blk])
                # batchloc broadcast down partitions (stride-0 DMA read)
                bcb = pb.tile([P, npb], f16, tag="bcb")
                nc.gpsimd.dma_start(
                    out=bcb[:], in_=blr_ext[blk:blk + 1].to_broadcast((P, npb)))
                # q1g[g, d] = (v_n_blk @ W1_w.T)[g, d]  (biases folded later)
                q1g_ps = psBlk.tile([P, P], f32, tag="blkmm", space="PSUM")
                nc.tensor.matmul(out=q1g_ps[:], lhsT=vnt[:, gsl], rhs=w1t[:],
                                 start=True, stop=True)
                q1g = blkp.tile([P, P], f16, tag="q1g")
                nc.vector.tensor_copy(out=q1g[:], in_=q1g_ps[:])

                sg_ps = psSg.tile([P, P], f32, tag="sg", space="PSUM")
                mm_i = 0
                n_mm = sum(min(ST, ntpb - ST * s) for s in range(nst))
                for st in range(nst):
                    nsub = min(ST, ntpb - ST * st)
                    w = nsub * P
                    ssl = slice(st * ST * P, st * ST * P + w)  # cols in block
                    # S^T[g, n] = (batchloc[n] == g)   [one op, 512 wide]
                    StT = pool.tile([P, ST * P], f16, tag="StT")
                    nc.vector.tensor_scalar(out=StT[:, :w], in0=bcb[:, ssl],
                                            scalar1=iota_col[:], scalar2=None,
                                            op0=EQ)
                    # S[n, g] per 128-node subtile
                    S_st = pool.tile([P, ST * P], f16, tag="S")
                    for c in range(nsub):
                        csl = slice(c * P, (c + 1) * P)
                        nc.vector.tensor_scalar(
                            out=S_st[:, csl], in0=iota_row[:],
                            scalar1=blc[:, st * ST + c:st * ST + c + 1],
                            scalar2=None, op0=EQ)
                    # pre^T[d, n] = W2 @ x^T + q1g^T-expand   (+b12 in ACT)
                    pre_ps = psPre.tile([P, ST * P], f32, tag="pre", space="PSUM")
                    nc.tensor.matmul(out=pre_ps[:, :w], lhsT=w2t[:],
                                     rhs=xtp[:, ssl], start=True, stop=False)
                    nc.tensor.matmul(out=pre_ps[:, :w], lhsT=q1g[:],
                                     rhs=StT[:, :w], start=False, stop=True)
                    sigT = pool.tile([P, ST * P], f16, tag="sigT")
                    nc.scalar.activation(out=sigT[:, :w], in_=pre_ps[:, :w],
                                         func=Sig, bias=b12c[:])
                    # alpha[n] = sig @ q_w.T (+ q_b in copy)
                    al_ps = psAl.tile([P, ST], f32, tag="al", space="PSUM")
                    for c in range(nsub):
                        csl = slice(c * P, (c + 1) * P)
                        nc.tensor.matmul(out=al_ps[:, c:c + 1],
                                         lhsT=sigT[:, csl], rhs=qwt[:],
                                         start=True, stop=True)
                    al = pool.tile([P, ST], f32, tag="al")
                    nc.vector.tensor_scalar_add(out=al[:, :nsub],
                                                in0=al_ps[:, :nsub],
                                                scalar1=qbc[:])
                    # aS = alpha * S ; s_g^T[d, g] += x^T-reduce via aS
                    aS = pool.tile([P, ST * P], f16, tag="aS")
                    for c in range(nsub):
                        csl = slice(c * P, (c + 1) * P)
                        nc.vector.tensor_scalar_mul(
                            out=aS[:, csl], in0=S_st[:, csl],
                            scalar1=al[:, c:c + 1])
                        nc.tensor.matmul(
                            out=sg_ps[:],
                            lhsT=xpk[:, st * ST * P + c * P:st * ST * P + (c + 1) * P],
                            rhs=aS[:, csl],
                            start=(mm_i == 0), stop=(mm_i == n_mm - 1))
                        mm_i += 1

                sg_sb = blkp.tile([P, P], f16, tag="sgsb")
                nc.vector.tensor_copy(out=sg_sb[:], in_=sg_ps[:])
                # s_h^T[d, g] = W3a @ v_n^T + W3b @ s_g^T  (+W3_b in copy)
                sh_ps = psBlk.tile([P, P], f32, tag="blkmm", space="PSUM")
                nc.tensor.matmul(out=sh_ps[:], lhsT=w3at[:], rhs=vnt[:, gsl],
                                 start=True, stop=False)
                nc.tensor.matmul(out=sh_ps[:], lhsT=w3bt[:], rhs=sg_sb[:],
                                 start=False, stop=True)
                nc.vector.tensor_scalar_add(out=shT[:, gsl], in0=sh_ps[:],
                                            scalar1=w3bc[:])

        if p2:
            # ------------- phase 2: scores = s_h @ item_weight^T ----------
            for g in range(ngrp):
                gvsl = slice(g * W, (g + 1) * W)
                itw = p2i.tile([D, W], f16, tag="itw")
                nc.gpsimd.dma_start(out=itw[:], in_=itwt_ext[:, gvsl])
                for blk in range(NBLK):
                    gsl = slice(blk * P, (blk + 1) * P)
                    sc = p2o.tile([P, W], f16, tag="scsb")
                    for s in range(VG):
                        sc_ps = ps2.tile([P, VT], f32, tag="sc", space="PSUM")
                        nc.tensor.matmul(out=sc_ps[:],
                                         lhsT=shT[:, gsl],
                                         rhs=itw[:, s * VT:(s + 1) * VT],
                                         start=True, stop=True)
                        if not do_cp and s > 0:
                            continue
                        if s % 2 == 0:
                            nc.vector.tensor_copy(out=sc[:, s * VT:(s + 1) * VT],
                                                  in_=sc_ps[:])
                        else:
                            nc.scalar.copy(out=sc[:, s * VT:(s + 1) * VT],
                                           in_=sc_ps[:])
                    if do_st:
                        # split stores across the SP HWDGE ring and the
                        # gpsimd SWDGE queue -> two parallel DMA paths
                        eng = nc.sync if (g * NBLK + blk) % 2 == 0 else nc.gpsimd
                        eng.dma_start(out=y_ext[blk * P:(blk + 1) * P, gvsl],
                                      in_=sc[:])


def prep_inputs(session_embedding, item_weight, W1_w, W1_b, W2_w, W2_b,
                q_w, q_b, W3_w, W3_b, batch, num_graphs):
    """Host-side sharding/layout. Returns (in_maps, ntpb, vpad, V)."""
    x = np.asarray(session_embedding, dtype=np.float32)
    itw = np.asarray(item_weight, dtype=np.float32)
    batch = np.asarray(batch).astype(np.int64)
    B = int(num_graphs)
    N, d = x.shape
    V = itw.shape[0]
    assert d == D and B == NCORES * BC, (d, B)

    counts = np.bincount(batch, minlength=B)
    assert counts.min() >= 1, "every graph must be non-empty"
    starts = np.zeros(B + 1, np.int64)
    np.cumsum(counts, out=starts[1:])
    assert starts[-1] == N
    last_idx = starts[1:] - 1
    v_n = x[last_idx]                                   # [B, D]

    blk_cnt = starts[P::P] - starts[:-P:P].reshape(-1)  # [B//P]
    ntpb = int(-(-blk_cnt.max() // P))                  # ceil
    npb = ntpb * P

    vpad = -(-V // (VT * VG)) * (VT * VG)
    itwT = np.zeros((D, vpad), np.float16)
    itwT[:, :V] = itw.T.astype(np.float16)

    w1t = np.ascontiguousarray(np.asarray(W1_w, np.float32).T).astype(np.float16)
    w2t = np.ascontiguousarray(np.asarray(W2_w, np.float32).T).astype(np.float16)
    W3 = np.asarray(W3_w, np.float32)
    w3at = np.ascontiguousarray(W3[:, :D].T).astype(np.float16)
    w3bt = np.ascontiguousarray(W3[:, D:].T).astype(np.float16)
    b12c = (np.asarray(W1_b, np.float32) + np.asarray(W2_b, np.float32)
            ).reshape(P, 1).copy()
    w3bc = np.asarray(W3_b, np.float32).reshape(P, 1).copy()
    qwt = np.ascontiguousarray(
        np.asarray(q_w, np.float32).reshape(1, D).T).astype(np.float16)
    qbc = np.full((P, 1), np.float32(np.asarray(q_b).reshape(())), np.float32)

    xh = x.astype(np.float16)
    in_maps = []
    for c in range(NCORES):
        xpad = np.zeros((NBLK, npb, D), np.float16)
        bl = np.zeros((NBLK, P, ntpb), np.float32)
        blr = np.zeros((NBLK, npb), np.float16)
        for b in range(NBLK):
            glo = c * BC + b * P
            s, e = int(starts[glo]), int(starts[glo + P])
            cnt = e - s
            assert cnt <= npb
            xpad[b, :cnt] = xh[s:e]
            locp = np.zeros(npb, np.float32)
            locp[:cnt] = (batch[s:e] - glo).astype(np.float32)
            bl[b] = locp.reshape(ntpb, P).T
            blr[b] = locp.astype(np.float16)
        # packed node-row: xpk[:, blk*npb + t*128 + j][i] = x_pad[blk, t*128+i, j]
        xpk = np.ascontiguousarray(
            xpad.reshape(NBLK, ntpb, P, D).transpose(2, 0, 1, 3).reshape(P, NBLK * npb))
        # feature-row transposed: xtp[:, blk*npb + n] = x_pad[blk, n, :]
        xtp = np.ascontiguousarray(
            xpad.transpose(2, 0, 1).reshape(P, NBLK * npb))
        vntc = np.ascontiguousarray(v_n[c * BC:(c + 1) * BC].T).astype(np.float16)
        im = dict(
            xpk=xpk, xtp=xtp, blc=np.ascontiguousarray(bl),
            blr=np.ascontiguousarray(blr), vnt=vntc,
            w1t=w1t, w2t=w2t, w3at=w3at, w3bt=w3bt,
            b12c=b12c, w3bc=w3bc, qwt=qwt, qbc=qbc, itwt=itwT)
        in_maps.append(im)
    return in_maps, ntpb, vpad, V


_NC_CACHE = {}


def get_nc(ntpb, vpad, repeat=1, phase="both"):
    key = (ntpb, vpad, repeat, phase)
    if key not in _NC_CACHE:
        _NC_CACHE[key] = build_nc(ntpb, vpad, repeat, phase)
    return _NC_CACHE[key]


def kernel(**inputs):
    from concourse.bass_utils import run_bass_kernel_spmd

    in_maps, ntpb, vpad, V = prep_inputs(**inputs)
    nc = get_nc(ntpb, vpad)
    res = run_bass_kernel_spmd(nc, in_maps, core_ids=list(range(NCORES)))
    B = int(inputs["num_graphs"])
    y = np.empty((B, V), np.float32)
    for c in range(NCORES):
        y[c * BC:(c + 1) * BC] = res.results[c]["y"][:, :V].astype(np.float32)
    return y

